# revision 7
# baseline (speedup 1.0000x reference)
"""Trainium2 Bass kernel for nn_ColorConsistencyLoss (segment_reduce).

loss = 0.7 * mean_CE(log_softmax(output), target) + 0.3 * sigmoid(sum_l,c std(img_c * mask_l))

v4 strategy (8 NeuronCores, data-parallel over pixels; all heavy math bf16).

Host packs, per group g of 128 pixels, a stationary block
comb[128, 106] = [o (100 logits) | img (3) | img^2 (3)] (bf16). Per group a
bf16 one-hot OH[p, l] = (target_p == l) is built with a flat tensor_scalar
is_equal (DVE 4x packed mode; flat 2-dim APs only -- on real TRN2 silicon
multi-dim strided APs drop DVE to 1x, which the cost model misses). The
one-hot work is split DVE / gpsimd to balance engines. One PE matmul per
group

    PSUM[0:106, 0:100] += comb_g^T @ OH_g

accumulated over all 1024 groups yields at once
  rows 0:100   -> trace = CE gather term sum_p o[p, t_p]
  rows 100:106 -> per-label (sum img, sum img^2) moments  [6, 100]
so CE needs no per-pixel gather product; host extracts trace/moments from
one [106,100] f32 output.

ACT computes exp(o) reading comb group-major but writing expo LABEL-major
(expo[p, l*32+j]): the strided read costs ACT nothing (1x all dtypes) and
makes the softmax-denominator reduction flat: two flat bf16 tensor_tensor
folds (labels 0:50 + 50:100, then 0:25 + 25:50; both 2x packed) followed by
one short strided tensor_reduce (1x but only 800 elems). ACT Ln(s) with
accum_out gives sum_p log s_p. Host: CE = (sum lse - trace)/HW,
var/std/sigmoid on the moments (loss2's sigmoid saturates at 1.0, so moment
precision is uncritical).

comb DMA is plain HWDGE (host ships bf16, halving HBM bytes vs f32+cast);
each partition reads one contiguous 6.8KB run per macro-tile of 32 groups.
"""

import contextlib
import sys

for _p in ("/opt/trn_rl_repo", "/opt/trn_rl_repo/concourse"):
    if _p not in sys.path:
        sys.path.insert(0, _p)

import numpy as np

import concourse.bacc as bacc
import concourse.tile as tile
from concourse import mybir
from concourse.bass_utils import run_bass_kernel_spmd

# ---------------------------------------------------------------- constants
HW = 1048576          # total pixels
L = 100               # num labels (softmax width)
K = 106               # comb row width: 100 logits + img(3) + img^2(3)
LP = 128              # iota width
N_CORES = 8
PIX_PER_CORE = HW // N_CORES          # 131072
GPM = 32              # groups (of 128 pixels) per macro-tile
PIX_PER_MACRO = 128 * GPM             # 4096
N_MACROS = PIX_PER_CORE // PIX_PER_MACRO   # 32
ALPHA_SAL = 0.3

F32 = mybir.dt.float32
BF16 = mybir.dt.bfloat16
NP_BF16 = mybir.dt.np(BF16)


def build_nc(
    n_macros: int = N_MACROS,
    gpm: int = GPM,
    repeats: int = 1,
    n_oh_dve: int = 16,
    io_bufs: int = 4,
    oh_bufs: int = 4,
    ex_bufs: int = 4,
    staggered: bool = True,
):
    """Build the single-core Bass program (same program runs SPMD on all cores).

    repeats > 1 wraps the compute in an on-device For_i loop (benchmarking).
    n_oh_dve: per-macro one-hot groups built on DVE; the rest go to gpsimd.
    """
    n_groups_total = gpm * n_macros
    half = gpm * 50   # flat fold sizes in the label-major expo layout

    nc = bacc.Bacc("TRN2")

    comb_d = nc.dram_tensor("comb", [n_macros * 128, gpm * K], BF16, kind="ExternalInput")
    iota_d = nc.dram_tensor("iota", [128, LP], BF16, kind="ExternalInput")
    tgtf_d = nc.dram_tensor("tgtf", [128, n_groups_total], F32, kind="ExternalInput")
    lse_d = nc.dram_tensor("lse_out", [128, 1], F32, kind="ExternalOutput")
    st_d = nc.dram_tensor("st_out", [K, L], F32, kind="ExternalOutput")

    comb_view = comb_d[:, :].rearrange("(m p) e -> m p e", p=128)

    with tile.TileContext(nc) as tc:
        with (
            tc.tile_pool(name="consts", bufs=1) as cpool,
            tc.tile_pool(name="cbuf", bufs=io_bufs) as combpool,
            tc.tile_pool(name="ebuf", bufs=ex_bufs) as epool,
            tc.tile_pool(name="ohbuf", bufs=oh_bufs) as ohpool,
            tc.tile_pool(name="fbuf", bufs=2) as fpool,
            tc.tile_pool(name="psum", bufs=1, space="PSUM") as ppool,
        ):
            iota_sb = cpool.tile([128, LP], BF16)
            nc.sync.dma_start(out=iota_sb, in_=iota_d[:, :])
            tgt_sb = cpool.tile([128, n_groups_total], F32)
            nc.sync.dma_start(out=tgt_sb, in_=tgtf_d[:, :])
            s_sb = cpool.tile([128, n_groups_total], BF16)
            # Warm-up: each compute engine observes both const DMAs once, so
            # loop instructions don't each accumulate waits on the DMA sems.
            wu_v = cpool.tile([128, 1], F32)
            nc.vector.tensor_scalar(
                out=wu_v, in0=iota_sb[:, 0:1], scalar1=tgt_sb[:, 0:1],
                scalar2=None, op0=mybir.AluOpType.mult,
            )
            wu_s = cpool.tile([128, 1], BF16)
            nc.scalar.copy(out=wu_s, in_=iota_sb[:, 0:1])
            wu_s2 = cpool.tile([128, 1], F32)
            nc.scalar.copy(out=wu_s2, in_=tgt_sb[:, 0:1])
            wu_p = cpool.tile([128, 1], F32)
            nc.gpsimd.tensor_scalar(
                out=wu_p, in0=iota_sb[:, 0:1], scalar1=tgt_sb[:, 0:1],
                scalar2=None, op0=mybir.AluOpType.add,
            )

            st_ps = ppool.tile([K, L], F32)

            loop_cm = (
                tc.For_i(0, repeats, 1, staggered_reset=staggered)
                if repeats > 1
                else contextlib.nullcontext()
            )
            with loop_cm:
                for m in range(n_macros):
                    comb_t = combpool.tile([128, gpm * K], BF16, tag="comb")
                    nc.sync.dma_start(out=comb_t, in_=comb_view[m])
                    comb_3d = comb_t.rearrange("p (j k) -> p j k", k=K)

                    # --- one-hots (flat TS, DVE/gpsimd split) -------------
                    oh = ohpool.tile([128, gpm * L], BF16, tag="oh")
                    for j in range(gpm):
                        gidx = m * gpm + j
                        teng = nc.vector if j < n_oh_dve else nc.gpsimd
                        teng.tensor_scalar(
                            out=oh[:, j * L : (j + 1) * L],
                            in0=iota_sb[:, 0:L],
                            scalar1=tgt_sb[:, gidx : gidx + 1],
                            scalar2=None,
                            op0=mybir.AluOpType.is_equal,
                        )

                    # --- exp, written label-major: expo[p, l*gpm + j] -----
                    expo = epool.tile([128, gpm * L], BF16, tag="expo")
                    nc.scalar.activation(
                        out=expo.rearrange("p (l j) -> p l j", j=gpm),
                        in_=comb_3d[:, :, 0:L].rearrange("p j k -> p k j"),
                        func=mybir.ActivationFunctionType.Exp,
                    )
                    # --- denominators: flat folds + short reduce ----------
                    h1 = fpool.tile([128, half], BF16, tag="h1")
                    nc.vector.tensor_tensor(
                        out=h1, in0=expo[:, 0:half], in1=expo[:, half : 2 * half],
                        op=mybir.AluOpType.add,
                    )
                    h2 = fpool.tile([128, half // 2], BF16, tag="h2")
                    nc.vector.tensor_tensor(
                        out=h2, in0=h1[:, 0 : half // 2], in1=h1[:, half // 2 : half],
                        op=mybir.AluOpType.add,
                    )
                    with nc.allow_low_precision(
                        reason="bf16 softmax denominators; CE error averages "
                        "over 1M pixels"
                    ):
                        nc.vector.tensor_reduce(
                            out=s_sb[:, m * gpm : (m + 1) * gpm],
                            in_=h2.rearrange("p (l j) -> p j l", j=gpm),
                            axis=mybir.AxisListType.X,
                            op=mybir.AluOpType.add,
                        )

                    # --- gather + moments on PE ---------------------------
                    for j in range(gpm):
                        gidx = m * gpm + j
                        nc.tensor.matmul(
                            st_ps,
                            lhsT=comb_t[:, j * K : (j + 1) * K],
                            rhs=oh[:, j * L : (j + 1) * L],
                            start=gidx == 0,
                            stop=gidx == n_groups_total - 1,
                        )

                # --- finals ------------------------------------------------
                lnj = cpool.tile([128, n_groups_total], BF16)
                lse_sb = cpool.tile([128, 1], F32)
                nc.scalar.activation(
                    out=lnj,
                    in_=s_sb,
                    func=mybir.ActivationFunctionType.Ln,
                    accum_out=lse_sb,
                )
                nc.sync.dma_start(out=lse_d[:, :], in_=lse_sb)
                st_sb = cpool.tile([K, L], F32)
                nc.vector.tensor_copy(out=st_sb, in_=st_ps)
                nc.sync.dma_start(out=st_d[:, :], in_=st_sb)

    nc.compile()  # bacc lowering: splits >1-wait instructions for the TRN2 ISA
    return nc


def make_in_map(o_slice, tgt_slice, img_slice, n_macros: int = N_MACROS, gpm: int = GPM):
    """Host-side pre-layout for one core.

    Pixel q = m*(128*gpm) + p*gpm + j  ->  macro m, partition p, group j.
    comb[m*128+p, j*K + 0:100]   = o[q]        (bf16)
    comb[m*128+p, j*K + 100:103] = img[q]      (bf16)
    comb[m*128+p, j*K + 103:106] = img[q]^2    (bf16)
    tgtf[p, m*gpm+j] = target[q]               (f32)
    """
    n_pix = 128 * gpm * n_macros
    assert o_slice.shape == (n_pix, L)

    o4 = np.asarray(o_slice, dtype=np.float32).reshape(n_macros, 128, gpm, L)
    img4 = np.asarray(img_slice, dtype=np.float32).reshape(n_macros, 128, gpm, 3)
    comb = np.empty((n_macros, 128, gpm, K), dtype=np.float32)
    comb[..., 0:L] = o4
    comb[..., L : L + 3] = img4
    comb[..., L + 3 : K] = img4 * img4

    t = np.asarray(tgt_slice).reshape(n_macros, 128, gpm)
    tgt_pre = np.ascontiguousarray(t.transpose(1, 0, 2)).reshape(128, n_macros * gpm)

    iota = np.broadcast_to(np.arange(LP, dtype=np.float32), (128, LP))
    return {
        "comb": comb.astype(NP_BF16).reshape(n_macros * 128, gpm * K),
        "iota": np.ascontiguousarray(iota.astype(NP_BF16)),
        "tgtf": np.ascontiguousarray(tgt_pre.astype(np.float32)),
    }


def finalize(results, n_pix_total=HW):
    """Combine per-core partial results (host-side unshard) into the scalar loss."""
    lse_sum = 0.0
    gather_sum = 0.0
    s1 = np.zeros((L, 3), dtype=np.float64)
    s2 = np.zeros((L, 3), dtype=np.float64)
    for r in results:
        lse_sum += float(np.sum(np.asarray(r["lse_out"], dtype=np.float64)))
        st = np.asarray(r["st_out"], dtype=np.float64)  # [106, 100]
        gather_sum += float(np.trace(st[0:L, 0:L]))
        s1 += st[L : L + 3, :].T
        s2 += st[L + 3 : K, :].T
    loss1 = (lse_sum - gather_sum) / n_pix_total
    mean = s1 / n_pix_total
    var = np.maximum(s2 / n_pix_total - mean * mean, 0.0)
    std_all = float(np.sum(np.sqrt(var)))
    loss2 = 1.0 / (1.0 + np.exp(-std_all))
    return np.float32((1.0 - ALPHA_SAL) * loss1 + ALPHA_SAL * loss2)


_NC_CACHE = {}


def _get_nc():
    if "nc" not in _NC_CACHE:
        _NC_CACHE["nc"] = build_nc()
    return _NC_CACHE["nc"]


def kernel(output, target, img):
    output = np.asarray(output, dtype=np.float32)
    target = np.asarray(target)
    img = np.asarray(img, dtype=np.float32)
    assert output.shape == (HW, L)
    img_flat = img.reshape(HW, 3)

    in_maps = []
    for c in range(N_CORES):
        lo, hi = c * PIX_PER_CORE, (c + 1) * PIX_PER_CORE
        in_maps.append(
            make_in_map(output[lo:hi], target[lo:hi], img_flat[lo:hi])
        )

    nc = _get_nc()
    res = run_bass_kernel_spmd(nc, in_maps, core_ids=list(range(N_CORES)))
    return finalize(res.results)


if __name__ == "__main__":
    nc = build_nc(n_macros=1)
    print("built ok:", len(nc.inst_map), "instructions")


# revision 9
# speedup vs baseline: 1.9847x; 1.9847x over previous
"""Trainium2 Bass kernel for nn_ColorConsistencyLoss (segment_reduce).

loss = 0.7 * mean_CE(log_softmax(output), target) + 0.3 * sigmoid(sum_l,c std(img_c * mask_l))

v4 strategy (8 NeuronCores, data-parallel over pixels; all heavy math bf16).

Host packs, per group g of 128 pixels, a stationary block
comb[128, 106] = [o (100 logits) | img (3) | img^2 (3)] (bf16). Per group a
bf16 one-hot OH[p, l] = (target_p == l) is built with a flat tensor_scalar
is_equal (DVE 4x packed mode; flat 2-dim APs only -- on real TRN2 silicon
multi-dim strided APs drop DVE to 1x, which the cost model misses). The
one-hot work is split DVE / gpsimd to balance engines. One PE matmul per
group

    PSUM[0:106, 0:100] += comb_g^T @ OH_g

accumulated over all 1024 groups yields at once
  rows 0:100   -> trace = CE gather term sum_p o[p, t_p]
  rows 100:106 -> per-label (sum img, sum img^2) moments  [6, 100]
so CE needs no per-pixel gather product; host extracts trace/moments from
one [106,100] f32 output.

ACT computes exp(o) reading comb group-major but writing expo LABEL-major
(expo[p, l*32+j]): the strided read costs ACT nothing (1x all dtypes) and
makes the softmax-denominator reduction flat: two flat bf16 tensor_tensor
folds (labels 0:50 + 50:100, then 0:25 + 25:50; both 2x packed) followed by
one short strided tensor_reduce (1x but only 800 elems). ACT Ln(s) with
accum_out gives sum_p log s_p. Host: CE = (sum lse - trace)/HW,
var/std/sigmoid on the moments (loss2's sigmoid saturates at 1.0, so moment
precision is uncritical).

comb DMA is plain HWDGE (host ships bf16, halving HBM bytes vs f32+cast);
each partition reads one contiguous 6.8KB run per macro-tile of 32 groups.
"""

import contextlib
import sys

for _p in ("/opt/trn_rl_repo", "/opt/trn_rl_repo/concourse"):
    if _p not in sys.path:
        sys.path.insert(0, _p)

import numpy as np

import concourse.bacc as bacc
import concourse.tile as tile
from concourse import mybir
from concourse.bass_utils import run_bass_kernel_spmd

# ---------------------------------------------------------------- constants
HW = 1048576          # total pixels
L = 100               # num labels (softmax width)
K = 106               # comb row width: 100 logits + img(3) + img^2(3)
LP = 128              # iota width
N_CORES = 8
PIX_PER_CORE = HW // N_CORES          # 131072
GPM = 32              # groups (of 128 pixels) per macro-tile
PIX_PER_MACRO = 128 * GPM             # 4096
N_MACROS = PIX_PER_CORE // PIX_PER_MACRO   # 32
ALPHA_SAL = 0.3

F32 = mybir.dt.float32
BF16 = mybir.dt.bfloat16
NP_BF16 = mybir.dt.np(BF16)


def build_nc(
    n_macros: int = N_MACROS,
    gpm: int = GPM,
    repeats: int = 1,
    n_oh_dve: int = 23,
    f2_pool: bool = True,
    io_bufs: int = 4,
    oh_bufs: int = 4,
    ex_bufs: int = 4,
    staggered: bool = True,
):
    """Build the single-core Bass program (same program runs SPMD on all cores).

    repeats > 1 wraps the compute in an on-device For_i loop (benchmarking).
    n_oh_dve: per-macro one-hot groups built on DVE; the rest go to gpsimd.
    """
    n_groups_total = gpm * n_macros
    half = gpm * 50   # flat fold sizes in the label-major expo layout

    nc = bacc.Bacc("TRN2")

    comb_d = nc.dram_tensor("comb", [n_macros * 128, gpm * K], BF16, kind="ExternalInput")
    iota_d = nc.dram_tensor("iota", [128, LP], BF16, kind="ExternalInput")
    tgtf_d = nc.dram_tensor("tgtf", [128, n_groups_total], F32, kind="ExternalInput")
    lse_d = nc.dram_tensor("lse_out", [128, 1], F32, kind="ExternalOutput")
    st_d = nc.dram_tensor("st_out", [K, L], F32, kind="ExternalOutput")

    comb_view = comb_d[:, :].rearrange("(m p) e -> m p e", p=128)

    with tile.TileContext(nc) as tc:
        with (
            tc.tile_pool(name="consts", bufs=1) as cpool,
            tc.tile_pool(name="cbuf", bufs=io_bufs) as combpool,
            tc.tile_pool(name="ebuf", bufs=ex_bufs) as epool,
            tc.tile_pool(name="ohbuf", bufs=oh_bufs) as ohpool,
            tc.tile_pool(name="fbuf", bufs=2) as fpool,
            tc.tile_pool(name="psum", bufs=1, space="PSUM") as ppool,
        ):
            iota_sb = cpool.tile([128, LP], BF16)
            nc.sync.dma_start(out=iota_sb, in_=iota_d[:, :])
            tgt_sb = cpool.tile([128, n_groups_total], F32)
            nc.sync.dma_start(out=tgt_sb, in_=tgtf_d[:, :])
            s_sb = cpool.tile([128, n_groups_total], BF16)
            # Warm-up: each compute engine observes both const DMAs once, so
            # loop instructions don't each accumulate waits on the DMA sems.
            wu_v = cpool.tile([128, 1], F32)
            nc.vector.tensor_scalar(
                out=wu_v, in0=iota_sb[:, 0:1], scalar1=tgt_sb[:, 0:1],
                scalar2=None, op0=mybir.AluOpType.mult,
            )
            wu_s = cpool.tile([128, 1], BF16)
            nc.scalar.copy(out=wu_s, in_=iota_sb[:, 0:1])
            wu_s2 = cpool.tile([128, 1], F32)
            nc.scalar.copy(out=wu_s2, in_=tgt_sb[:, 0:1])
            wu_p = cpool.tile([128, 1], F32)
            nc.gpsimd.tensor_scalar(
                out=wu_p, in0=iota_sb[:, 0:1], scalar1=tgt_sb[:, 0:1],
                scalar2=None, op0=mybir.AluOpType.add,
            )

            st_ps = ppool.tile([K, L], F32)

            loop_cm = (
                tc.For_i(0, repeats, 1, staggered_reset=staggered)
                if repeats > 1
                else contextlib.nullcontext()
            )
            with loop_cm:
                for m in range(n_macros):
                    comb_t = combpool.tile([128, gpm * K], BF16, tag="comb")
                    nc.sync.dma_start(out=comb_t, in_=comb_view[m])
                    comb_3d = comb_t.rearrange("p (j k) -> p j k", k=K)

                    # --- one-hots (flat TS, DVE/gpsimd split) -------------
                    oh = ohpool.tile([128, gpm * L], BF16, tag="oh")
                    for j in range(gpm):
                        gidx = m * gpm + j
                        teng = nc.vector if j < n_oh_dve else nc.gpsimd
                        teng.tensor_scalar(
                            out=oh[:, j * L : (j + 1) * L],
                            in0=iota_sb[:, 0:L],
                            scalar1=tgt_sb[:, gidx : gidx + 1],
                            scalar2=None,
                            op0=mybir.AluOpType.is_equal,
                        )

                    # --- exp in two label-half tiles (all APs keep >=50-elem
                    # contiguous inner runs; ACT is 1x but hates small-stride)
                    ex_a = epool.tile([128, half], BF16, tag="ex_a")
                    ex_b = epool.tile([128, half], BF16, tag="ex_b")
                    nc.scalar.activation(
                        out=ex_a.rearrange("p (j e) -> p j e", e=50),
                        in_=comb_3d[:, :, 0:50],
                        func=mybir.ActivationFunctionType.Exp,
                    )
                    nc.scalar.activation(
                        out=ex_b.rearrange("p (j e) -> p j e", e=50),
                        in_=comb_3d[:, :, 50:L],
                        func=mybir.ActivationFunctionType.Exp,
                    )
                    # --- denominators: flat fold (DVE 2x), strided fold
                    # (gpsimd), short strided reduce (DVE 1x, 800 elems) ----
                    h1 = fpool.tile([128, half], BF16, tag="h1")
                    nc.vector.tensor_tensor(
                        out=h1, in0=ex_a, in1=ex_b, op=mybir.AluOpType.add,
                    )
                    h13 = h1.rearrange("p (j e) -> p j e", e=50)
                    h2 = fpool.tile([128, half // 2], BF16, tag="h2")
                    f2_eng = nc.gpsimd if f2_pool else nc.vector
                    f2_eng.tensor_tensor(
                        out=h2.rearrange("p (j e) -> p j e", e=25),
                        in0=h13[:, :, 0:25], in1=h13[:, :, 25:50],
                        op=mybir.AluOpType.add,
                    )
                    with nc.allow_low_precision(
                        reason="bf16 softmax denominators; CE error averages "
                        "over 1M pixels"
                    ):
                        nc.vector.tensor_reduce(
                            out=s_sb[:, m * gpm : (m + 1) * gpm],
                            in_=h2.rearrange("p (j e) -> p j e", e=25),
                            axis=mybir.AxisListType.X,
                            op=mybir.AluOpType.add,
                        )

                    # --- gather + moments on PE ---------------------------
                    for j in range(gpm):
                        gidx = m * gpm + j
                        nc.tensor.matmul(
                            st_ps,
                            lhsT=comb_t[:, j * K : (j + 1) * K],
                            rhs=oh[:, j * L : (j + 1) * L],
                            start=gidx == 0,
                            stop=gidx == n_groups_total - 1,
                        )

                # --- finals ------------------------------------------------
                lnj = cpool.tile([128, n_groups_total], BF16)
                lse_sb = cpool.tile([128, 1], F32)
                nc.scalar.activation(
                    out=lnj,
                    in_=s_sb,
                    func=mybir.ActivationFunctionType.Ln,
                    accum_out=lse_sb,
                )
                nc.sync.dma_start(out=lse_d[:, :], in_=lse_sb)
                st_sb = cpool.tile([K, L], F32)
                nc.vector.tensor_copy(out=st_sb, in_=st_ps)
                nc.sync.dma_start(out=st_d[:, :], in_=st_sb)

    nc.compile()  # bacc lowering: splits >1-wait instructions for the TRN2 ISA
    return nc


def make_in_map(o_slice, tgt_slice, img_slice, n_macros: int = N_MACROS, gpm: int = GPM):
    """Host-side pre-layout for one core.

    Pixel q = m*(128*gpm) + p*gpm + j  ->  macro m, partition p, group j.
    comb[m*128+p, j*K + 0:100]   = o[q]        (bf16)
    comb[m*128+p, j*K + 100:103] = img[q]      (bf16)
    comb[m*128+p, j*K + 103:106] = img[q]^2    (bf16)
    tgtf[p, m*gpm+j] = target[q]               (f32)
    """
    n_pix = 128 * gpm * n_macros
    assert o_slice.shape == (n_pix, L)

    o4 = np.asarray(o_slice, dtype=np.float32).reshape(n_macros, 128, gpm, L)
    img4 = np.asarray(img_slice, dtype=np.float32).reshape(n_macros, 128, gpm, 3)
    comb = np.empty((n_macros, 128, gpm, K), dtype=np.float32)
    comb[..., 0:L] = o4
    comb[..., L : L + 3] = img4
    comb[..., L + 3 : K] = img4 * img4

    t = np.asarray(tgt_slice).reshape(n_macros, 128, gpm)
    tgt_pre = np.ascontiguousarray(t.transpose(1, 0, 2)).reshape(128, n_macros * gpm)

    iota = np.broadcast_to(np.arange(LP, dtype=np.float32), (128, LP))
    return {
        "comb": comb.astype(NP_BF16).reshape(n_macros * 128, gpm * K),
        "iota": np.ascontiguousarray(iota.astype(NP_BF16)),
        "tgtf": np.ascontiguousarray(tgt_pre.astype(np.float32)),
    }


def finalize(results, n_pix_total=HW):
    """Combine per-core partial results (host-side unshard) into the scalar loss."""
    lse_sum = 0.0
    gather_sum = 0.0
    s1 = np.zeros((L, 3), dtype=np.float64)
    s2 = np.zeros((L, 3), dtype=np.float64)
    for r in results:
        lse_sum += float(np.sum(np.asarray(r["lse_out"], dtype=np.float64)))
        st = np.asarray(r["st_out"], dtype=np.float64)  # [106, 100]
        gather_sum += float(np.trace(st[0:L, 0:L]))
        s1 += st[L : L + 3, :].T
        s2 += st[L + 3 : K, :].T
    loss1 = (lse_sum - gather_sum) / n_pix_total
    mean = s1 / n_pix_total
    var = np.maximum(s2 / n_pix_total - mean * mean, 0.0)
    std_all = float(np.sum(np.sqrt(var)))
    loss2 = 1.0 / (1.0 + np.exp(-std_all))
    return np.float32((1.0 - ALPHA_SAL) * loss1 + ALPHA_SAL * loss2)


_NC_CACHE = {}


def _get_nc():
    if "nc" not in _NC_CACHE:
        _NC_CACHE["nc"] = build_nc()
    return _NC_CACHE["nc"]


def kernel(output, target, img):
    output = np.asarray(output, dtype=np.float32)
    target = np.asarray(target)
    img = np.asarray(img, dtype=np.float32)
    assert output.shape == (HW, L)
    img_flat = img.reshape(HW, 3)

    in_maps = []
    for c in range(N_CORES):
        lo, hi = c * PIX_PER_CORE, (c + 1) * PIX_PER_CORE
        in_maps.append(
            make_in_map(output[lo:hi], target[lo:hi], img_flat[lo:hi])
        )

    nc = _get_nc()
    res = run_bass_kernel_spmd(nc, in_maps, core_ids=list(range(N_CORES)))
    return finalize(res.results)


if __name__ == "__main__":
    nc = build_nc(n_macros=1)
    print("built ok:", len(nc.inst_map), "instructions")


# revision 11
# speedup vs baseline: 3.0234x; 1.5234x over previous
"""Trainium2 Bass kernel for nn_ColorConsistencyLoss (segment_reduce).

loss = 0.7 * mean_CE(log_softmax(output), target) + 0.3 * sigmoid(sum_l,c std(img_c * mask_l))

v4 strategy (8 NeuronCores, data-parallel over pixels; all heavy math bf16).

Host packs, per group g of 128 pixels, a stationary block
comb[128, 106] = [o (100 logits) | img (3) | img^2 (3)] (bf16). Per group a
bf16 one-hot OH[p, l] = (target_p == l) is built with a flat tensor_scalar
is_equal (DVE 4x packed mode; flat 2-dim APs only -- on real TRN2 silicon
multi-dim strided APs drop DVE to 1x, which the cost model misses). The
one-hot work is split DVE / gpsimd to balance engines. One PE matmul per
group

    PSUM[0:106, 0:100] += comb_g^T @ OH_g

accumulated over all 1024 groups yields at once
  rows 0:100   -> trace = CE gather term sum_p o[p, t_p]
  rows 100:106 -> per-label (sum img, sum img^2) moments  [6, 100]
so CE needs no per-pixel gather product; host extracts trace/moments from
one [106,100] f32 output.

ACT computes exp(o) reading comb group-major but writing expo LABEL-major
(expo[p, l*32+j]): the strided read costs ACT nothing (1x all dtypes) and
makes the softmax-denominator reduction flat: two flat bf16 tensor_tensor
folds (labels 0:50 + 50:100, then 0:25 + 25:50; both 2x packed) followed by
one short strided tensor_reduce (1x but only 800 elems). ACT Ln(s) with
accum_out gives sum_p log s_p. Host: CE = (sum lse - trace)/HW,
var/std/sigmoid on the moments (loss2's sigmoid saturates at 1.0, so moment
precision is uncritical).

comb DMA is plain HWDGE (host ships bf16, halving HBM bytes vs f32+cast);
each partition reads one contiguous 6.8KB run per macro-tile of 32 groups.
"""

import contextlib
import sys

for _p in ("/opt/trn_rl_repo", "/opt/trn_rl_repo/concourse"):
    if _p not in sys.path:
        sys.path.insert(0, _p)

import numpy as np

import concourse.bacc as bacc
import concourse.tile as tile
from concourse import mybir
from concourse.bass_utils import run_bass_kernel_spmd

# ---------------------------------------------------------------- constants
HW = 1048576          # total pixels
L = 100               # num labels (softmax width)
K = 106               # comb row width: 100 logits + img(3) + img^2(3)
LP = 128              # iota width
N_CORES = 8
PIX_PER_CORE = HW // N_CORES          # 131072
GPM = 32              # groups (of 128 pixels) per macro-tile
PIX_PER_MACRO = 128 * GPM             # 4096
N_MACROS = PIX_PER_CORE // PIX_PER_MACRO   # 32
ALPHA_SAL = 0.3

F32 = mybir.dt.float32
BF16 = mybir.dt.bfloat16
NP_BF16 = mybir.dt.np(BF16)


def build_nc(
    n_macros: int = N_MACROS,
    gpm: int = GPM,
    repeats: int = 1,
    n_oh_dve: int = 32,
    use_f2: bool = False,
    f1_pool: bool = True,
    io_bufs: int = 4,
    oh_bufs: int = 4,
    ex_bufs: int = 4,
    staggered: bool = True,
):
    """Build the single-core Bass program (same program runs SPMD on all cores).

    repeats > 1 wraps the compute in an on-device For_i loop (benchmarking).
    n_oh_dve: per-macro one-hot groups built on DVE; the rest go to gpsimd.
    """
    n_groups_total = gpm * n_macros
    half = gpm * 50   # flat fold sizes in the label-major expo layout

    nc = bacc.Bacc("TRN2")

    comb_d = nc.dram_tensor("comb", [n_macros * 128, gpm * K], BF16, kind="ExternalInput")
    iota_d = nc.dram_tensor("iota", [128, LP], BF16, kind="ExternalInput")
    tgtf_d = nc.dram_tensor("tgtf", [128, n_groups_total], F32, kind="ExternalInput")
    lse_d = nc.dram_tensor("lse_out", [128, 1], F32, kind="ExternalOutput")
    st_d = nc.dram_tensor("st_out", [K, L], F32, kind="ExternalOutput")

    comb_view = comb_d[:, :].rearrange("(m p) e -> m p e", p=128)

    with tile.TileContext(nc) as tc:
        with (
            tc.tile_pool(name="consts", bufs=1) as cpool,
            tc.tile_pool(name="cbuf", bufs=io_bufs) as combpool,
            tc.tile_pool(name="ebuf", bufs=ex_bufs) as epool,
            tc.tile_pool(name="ohbuf", bufs=oh_bufs) as ohpool,
            tc.tile_pool(name="fbuf", bufs=2) as fpool,
            tc.tile_pool(name="psum", bufs=1, space="PSUM") as ppool,
        ):
            iota_sb = cpool.tile([128, LP], BF16)
            nc.sync.dma_start(out=iota_sb, in_=iota_d[:, :])
            tgt_sb = cpool.tile([128, n_groups_total], F32)
            nc.sync.dma_start(out=tgt_sb, in_=tgtf_d[:, :])
            s_sb = cpool.tile([128, n_groups_total], BF16)
            # Warm-up: each compute engine observes both const DMAs once, so
            # loop instructions don't each accumulate waits on the DMA sems.
            wu_v = cpool.tile([128, 1], F32)
            nc.vector.tensor_scalar(
                out=wu_v, in0=iota_sb[:, 0:1], scalar1=tgt_sb[:, 0:1],
                scalar2=None, op0=mybir.AluOpType.mult,
            )
            wu_s = cpool.tile([128, 1], BF16)
            nc.scalar.copy(out=wu_s, in_=iota_sb[:, 0:1])
            wu_s2 = cpool.tile([128, 1], F32)
            nc.scalar.copy(out=wu_s2, in_=tgt_sb[:, 0:1])
            wu_p = cpool.tile([128, 1], F32)
            nc.gpsimd.tensor_scalar(
                out=wu_p, in0=iota_sb[:, 0:1], scalar1=tgt_sb[:, 0:1],
                scalar2=None, op0=mybir.AluOpType.add,
            )

            st_ps = ppool.tile([K, L], F32)

            loop_cm = (
                tc.For_i(0, repeats, 1, staggered_reset=staggered)
                if repeats > 1
                else contextlib.nullcontext()
            )
            with loop_cm:
                for m in range(n_macros):
                    comb_t = combpool.tile([128, gpm * K], BF16, tag="comb")
                    nc.sync.dma_start(out=comb_t, in_=comb_view[m])
                    comb_3d = comb_t.rearrange("p (j k) -> p j k", k=K)

                    # --- one-hots (flat TS, DVE/gpsimd split) -------------
                    oh = ohpool.tile([128, gpm * L], BF16, tag="oh")
                    for j in range(gpm):
                        gidx = m * gpm + j
                        teng = nc.vector if j < n_oh_dve else nc.gpsimd
                        teng.tensor_scalar(
                            out=oh[:, j * L : (j + 1) * L],
                            in0=iota_sb[:, 0:L],
                            scalar1=tgt_sb[:, gidx : gidx + 1],
                            scalar2=None,
                            op0=mybir.AluOpType.is_equal,
                        )

                    # --- exp in two label-half tiles (all APs keep >=50-elem
                    # contiguous inner runs; ACT is 1x but hates small-stride)
                    ex_a = epool.tile([128, half], BF16, tag="ex_a")
                    ex_b = epool.tile([128, half], BF16, tag="ex_b")
                    nc.scalar.activation(
                        out=ex_a.rearrange("p (j e) -> p j e", e=50),
                        in_=comb_3d[:, :, 0:50],
                        func=mybir.ActivationFunctionType.Exp,
                    )
                    nc.scalar.activation(
                        out=ex_b.rearrange("p (j e) -> p j e", e=50),
                        in_=comb_3d[:, :, 50:L],
                        func=mybir.ActivationFunctionType.Exp,
                    )
                    # --- denominators: flat fold (DVE 2x), strided fold
                    # (gpsimd), short strided reduce (DVE 1x, 800 elems) ----
                    h1 = fpool.tile([128, half], BF16, tag="h1")
                    f1_eng = nc.gpsimd if f1_pool else nc.vector
                    f1_eng.tensor_tensor(
                        out=h1, in0=ex_a, in1=ex_b, op=mybir.AluOpType.add,
                    )
                    h13 = h1.rearrange("p (j e) -> p j e", e=50)
                    if use_f2:
                        h2 = fpool.tile([128, half // 2], BF16, tag="h2")
                        nc.vector.tensor_tensor(
                            out=h2.rearrange("p (j e) -> p j e", e=25),
                            in0=h13[:, :, 0:25], in1=h13[:, :, 25:50],
                            op=mybir.AluOpType.add,
                        )
                        red_in = h2.rearrange("p (j e) -> p j e", e=25)
                    else:
                        red_in = h13
                    with nc.allow_low_precision(
                        reason="bf16 softmax denominators; CE error averages "
                        "over 1M pixels"
                    ):
                        nc.vector.tensor_reduce(
                            out=s_sb[:, m * gpm : (m + 1) * gpm],
                            in_=red_in,
                            axis=mybir.AxisListType.X,
                            op=mybir.AluOpType.add,
                        )

                    # --- gather + moments on PE ---------------------------
                    for j in range(gpm):
                        gidx = m * gpm + j
                        nc.tensor.matmul(
                            st_ps,
                            lhsT=comb_t[:, j * K : (j + 1) * K],
                            rhs=oh[:, j * L : (j + 1) * L],
                            start=gidx == 0,
                            stop=gidx == n_groups_total - 1,
                        )

                # --- finals ------------------------------------------------
                lnj = cpool.tile([128, n_groups_total], BF16)
                lse_sb = cpool.tile([128, 1], F32)
                nc.scalar.activation(
                    out=lnj,
                    in_=s_sb,
                    func=mybir.ActivationFunctionType.Ln,
                    accum_out=lse_sb,
                )
                nc.sync.dma_start(out=lse_d[:, :], in_=lse_sb)
                st_sb = cpool.tile([K, L], F32)
                nc.vector.tensor_copy(out=st_sb, in_=st_ps)
                nc.sync.dma_start(out=st_d[:, :], in_=st_sb)

    nc.compile()  # bacc lowering: splits >1-wait instructions for the TRN2 ISA
    return nc


def make_in_map(o_slice, tgt_slice, img_slice, n_macros: int = N_MACROS, gpm: int = GPM):
    """Host-side pre-layout for one core.

    Pixel q = m*(128*gpm) + p*gpm + j  ->  macro m, partition p, group j.
    comb[m*128+p, j*K + 0:100]   = o[q]        (bf16)
    comb[m*128+p, j*K + 100:103] = img[q]      (bf16)
    comb[m*128+p, j*K + 103:106] = img[q]^2    (bf16)
    tgtf[p, m*gpm+j] = target[q]               (f32)
    """
    n_pix = 128 * gpm * n_macros
    assert o_slice.shape == (n_pix, L)

    o4 = np.asarray(o_slice, dtype=np.float32).reshape(n_macros, 128, gpm, L)
    img4 = np.asarray(img_slice, dtype=np.float32).reshape(n_macros, 128, gpm, 3)
    comb = np.empty((n_macros, 128, gpm, K), dtype=np.float32)
    comb[..., 0:L] = o4
    comb[..., L : L + 3] = img4
    comb[..., L + 3 : K] = img4 * img4

    t = np.asarray(tgt_slice).reshape(n_macros, 128, gpm)
    tgt_pre = np.ascontiguousarray(t.transpose(1, 0, 2)).reshape(128, n_macros * gpm)

    iota = np.broadcast_to(np.arange(LP, dtype=np.float32), (128, LP))
    return {
        "comb": comb.astype(NP_BF16).reshape(n_macros * 128, gpm * K),
        "iota": np.ascontiguousarray(iota.astype(NP_BF16)),
        "tgtf": np.ascontiguousarray(tgt_pre.astype(np.float32)),
    }


def finalize(results, n_pix_total=HW):
    """Combine per-core partial results (host-side unshard) into the scalar loss."""
    lse_sum = 0.0
    gather_sum = 0.0
    s1 = np.zeros((L, 3), dtype=np.float64)
    s2 = np.zeros((L, 3), dtype=np.float64)
    for r in results:
        lse_sum += float(np.sum(np.asarray(r["lse_out"], dtype=np.float64)))
        st = np.asarray(r["st_out"], dtype=np.float64)  # [106, 100]
        gather_sum += float(np.trace(st[0:L, 0:L]))
        s1 += st[L : L + 3, :].T
        s2 += st[L + 3 : K, :].T
    loss1 = (lse_sum - gather_sum) / n_pix_total
    mean = s1 / n_pix_total
    var = np.maximum(s2 / n_pix_total - mean * mean, 0.0)
    std_all = float(np.sum(np.sqrt(var)))
    loss2 = 1.0 / (1.0 + np.exp(-std_all))
    return np.float32((1.0 - ALPHA_SAL) * loss1 + ALPHA_SAL * loss2)


_NC_CACHE = {}


def _get_nc():
    if "nc" not in _NC_CACHE:
        _NC_CACHE["nc"] = build_nc()
    return _NC_CACHE["nc"]


def kernel(output, target, img):
    output = np.asarray(output, dtype=np.float32)
    target = np.asarray(target)
    img = np.asarray(img, dtype=np.float32)
    assert output.shape == (HW, L)
    img_flat = img.reshape(HW, 3)

    in_maps = []
    for c in range(N_CORES):
        lo, hi = c * PIX_PER_CORE, (c + 1) * PIX_PER_CORE
        in_maps.append(
            make_in_map(output[lo:hi], target[lo:hi], img_flat[lo:hi])
        )

    nc = _get_nc()
    res = run_bass_kernel_spmd(nc, in_maps, core_ids=list(range(N_CORES)))
    return finalize(res.results)


if __name__ == "__main__":
    nc = build_nc(n_macros=1)
    print("built ok:", len(nc.inst_map), "instructions")


# revision 12
# speedup vs baseline: 6.8047x; 2.2506x over previous
"""Trainium2 Bass kernel for nn_ColorConsistencyLoss (segment_reduce).

loss = 0.7 * mean_CE(log_softmax(output), target) + 0.3 * sigmoid(sum_l,c std(img_c * mask_l))

Strategy (8 NeuronCores, data-parallel over pixels; all heavy math bf16):

Host packs, per group g of 128 pixels, a stationary block
comb[128, 106] = [o (100 logits) | img (3) | img^2 (3)] (bf16). Per macro of
32 groups the device builds a label-major one-hot
    oh_t[p, l, j] = (tgt[p, j] == l)
in ONE wide tensor_tensor is_equal: in0 broadcasts the 32 targets across the
label dim; in1 is a host-shipped iota-replicated constant. One PE matmul per
group

    PSUM[0:106, 0:100] += comb_g^T @ oh_t[:, :, j]      (strided rhs)

accumulated over all 1024 groups yields at once
  rows 0:100   -> trace = CE gather term sum_p o[p, t_p]
  rows 100:106 -> per-label (sum img, sum img^2) moments  [6, 100]
so CE needs no per-pixel gather product; host extracts trace/moments from
one [106,100] f32 output. (loss2's sigmoid saturates at ~1.0, so moment
precision is uncritical; all accuracy lives in the CE term, whose bf16
rounding averages out over 1M pixels.)

ACT computes exp(o) in one wide strided activation per macro. Softmax
denominators: DVE tensor_reduce has no packed mode, so one tensor_tensor
fold (labels 0:50 + 50:100) runs on gpsimd -- its only instruction per
macro; gpsimd per-instruction dispatch is ~1us so it gets exactly one big
op -- then a second fold and a short reduce on DVE. ACT Ln(s) with
accum_out gives sum_p log s_p. Host: CE = (sum lse - trace)/HW,
var/std/sigmoid on the moments.

comb DMA is plain HWDGE (host ships bf16, halving HBM bytes vs f32+cast);
each partition reads one contiguous 6.8KB run per macro.
"""

import contextlib
import sys

for _p in ("/opt/trn_rl_repo", "/opt/trn_rl_repo/concourse"):
    if _p not in sys.path:
        sys.path.insert(0, _p)

import numpy as np

import concourse.bacc as bacc
import concourse.tile as tile
from concourse import mybir
from concourse.bass_utils import run_bass_kernel_spmd

# ---------------------------------------------------------------- constants
HW = 1048576          # total pixels
L = 100               # num labels (softmax width)
K = 106               # comb row width: 100 logits + img(3) + img^2(3)
N_CORES = 8
PIX_PER_CORE = HW // N_CORES          # 131072
GPM = 32              # groups (of 128 pixels) per macro-tile
PIX_PER_MACRO = 128 * GPM             # 4096
N_MACROS = PIX_PER_CORE // PIX_PER_MACRO   # 32
ALPHA_SAL = 0.3

F32 = mybir.dt.float32
BF16 = mybir.dt.bfloat16
NP_BF16 = mybir.dt.np(BF16)


def build_nc(
    n_macros: int = N_MACROS,
    gpm: int = GPM,
    repeats: int = 1,
    oh_dve_labels: int = 100,
    fold1_pool: bool = True,
    io_bufs: int = 4,
    oh_bufs: int = 4,
    ex_bufs: int = 4,
    staggered: bool = True,
):
    """Build the single-core Bass program (same program runs SPMD on all cores).

    repeats > 1 wraps the compute in an on-device For_i loop (benchmarking).
    oh_dve_labels: one-hot label rows built on DVE; the rest go to gpsimd
    (gpsimd rejects the broadcast TT in neuronxcc, so keep this at 100).
    """
    n_groups_total = gpm * n_macros
    a = oh_dve_labels

    nc = bacc.Bacc("TRN2")

    comb_d = nc.dram_tensor("comb", [n_macros * 128, gpm * K], BF16, kind="ExternalInput")
    iotarep_d = nc.dram_tensor("iotarep", [128, L * gpm], BF16, kind="ExternalInput")
    tgtf_d = nc.dram_tensor("tgtf", [128, n_groups_total], BF16, kind="ExternalInput")
    lse_d = nc.dram_tensor("lse_out", [128, 1], F32, kind="ExternalOutput")
    st_d = nc.dram_tensor("st_out", [K, L], F32, kind="ExternalOutput")

    comb_view = comb_d[:, :].rearrange("(m p) e -> m p e", p=128)

    with tile.TileContext(nc) as tc:
        with (
            tc.tile_pool(name="consts", bufs=1) as cpool,
            tc.tile_pool(name="cbuf", bufs=io_bufs) as combpool,
            tc.tile_pool(name="ebuf", bufs=ex_bufs) as epool,
            tc.tile_pool(name="ohbuf", bufs=oh_bufs) as ohpool,
            tc.tile_pool(name="fbuf", bufs=2) as fpool,
            tc.tile_pool(name="psum", bufs=1, space="PSUM") as ppool,
        ):
            iotarep_sb = cpool.tile([128, L * gpm], BF16)
            nc.sync.dma_start(out=iotarep_sb, in_=iotarep_d[:, :])
            iotarep3 = iotarep_sb.rearrange("p (l j) -> p l j", j=gpm)
            tgt_sb = cpool.tile([128, n_groups_total], BF16)
            nc.sync.dma_start(out=tgt_sb, in_=tgtf_d[:, :])
            s_sb = cpool.tile([128, n_groups_total], BF16)
            # Warm-up: each compute engine observes both const DMAs once, so
            # loop instructions don't each accumulate waits on the DMA sems.
            wu_v = cpool.tile([128, 1], BF16)
            nc.vector.tensor_tensor(
                out=wu_v, in0=iotarep_sb[:, 0:1], in1=tgt_sb[:, 0:1],
                op=mybir.AluOpType.mult,
            )
            wu_s = cpool.tile([128, 1], BF16)
            nc.scalar.copy(out=wu_s, in_=iotarep_sb[:, 0:1])
            wu_s2 = cpool.tile([128, 1], BF16)
            nc.scalar.copy(out=wu_s2, in_=tgt_sb[:, 0:1])
            wu_p = cpool.tile([128, 1], BF16)
            nc.gpsimd.tensor_tensor(
                out=wu_p, in0=iotarep_sb[:, 0:1], in1=tgt_sb[:, 0:1],
                op=mybir.AluOpType.add,
            )

            st_ps = ppool.tile([K, L], F32)

            loop_cm = (
                tc.For_i(0, repeats, 1, staggered_reset=staggered)
                if repeats > 1
                else contextlib.nullcontext()
            )
            with loop_cm:
                for m in range(n_macros):
                    comb_t = combpool.tile([128, gpm * K], BF16, tag="comb")
                    nc.sync.dma_start(out=comb_t, in_=comb_view[m])
                    comb_3d = comb_t.rearrange("p (j k) -> p j k", k=K)

                    # --- one-hot (label-major), one wide DVE TT -----------
                    oh_t = ohpool.tile([128, L * gpm], BF16, tag="oh")
                    oh3 = oh_t.rearrange("p (l j) -> p l j", j=gpm)
                    tgt_b = (
                        tgt_sb[:, m * gpm : (m + 1) * gpm]
                        .unsqueeze(1)
                        .broadcast_to([128, L, gpm])
                    )
                    if a > 0:
                        nc.vector.tensor_tensor(
                            out=oh3[:, 0:a, :], in0=tgt_b[:, 0:a, :],
                            in1=iotarep3[:, 0:a, :], op=mybir.AluOpType.is_equal,
                        )
                    if a < L:
                        nc.gpsimd.tensor_tensor(
                            out=oh3[:, a:L, :], in0=tgt_b[:, a:L, :],
                            in1=iotarep3[:, a:L, :], op=mybir.AluOpType.is_equal,
                        )

                    # --- softmax denominators -----------------------------
                    expo = epool.tile([128, gpm * L], BF16, tag="expo")
                    expo3 = expo.rearrange("p (j e) -> p j e", e=L)
                    nc.scalar.activation(
                        out=expo3,
                        in_=comb_3d[:, :, 0:L],
                        func=mybir.ActivationFunctionType.Exp,
                    )
                    h1 = fpool.tile([128, gpm * 50], BF16, tag="h1")
                    h13 = h1.rearrange("p (j e) -> p j e", e=50)
                    fold1_eng = nc.gpsimd if fold1_pool else nc.vector
                    fold1_eng.tensor_tensor(
                        out=h13, in0=expo3[:, :, 0:50], in1=expo3[:, :, 50:100],
                        op=mybir.AluOpType.add,
                    )
                    h2 = fpool.tile([128, gpm * 25], BF16, tag="h2")
                    h23 = h2.rearrange("p (j e) -> p j e", e=25)
                    nc.vector.tensor_tensor(
                        out=h23, in0=h13[:, :, 0:25], in1=h13[:, :, 25:50],
                        op=mybir.AluOpType.add,
                    )
                    with nc.allow_low_precision(
                        reason="bf16 softmax denominators; CE error averages "
                        "over 1M pixels"
                    ):
                        nc.vector.tensor_reduce(
                            out=s_sb[:, m * gpm : (m + 1) * gpm],
                            in_=h23,
                            axis=mybir.AxisListType.X,
                            op=mybir.AluOpType.add,
                        )

                    # --- gather + moments on PE ---------------------------
                    for j in range(gpm):
                        gidx = m * gpm + j
                        nc.tensor.matmul(
                            st_ps,
                            lhsT=comb_t[:, j * K : (j + 1) * K],
                            rhs=oh3[:, :, j],
                            start=gidx == 0,
                            stop=gidx == n_groups_total - 1,
                        )

                # --- finals ------------------------------------------------
                lnj = cpool.tile([128, n_groups_total], BF16)
                lse_sb = cpool.tile([128, 1], F32)
                nc.scalar.activation(
                    out=lnj,
                    in_=s_sb,
                    func=mybir.ActivationFunctionType.Ln,
                    accum_out=lse_sb,
                )
                nc.sync.dma_start(out=lse_d[:, :], in_=lse_sb)
                st_sb = cpool.tile([K, L], F32)
                nc.vector.tensor_copy(out=st_sb, in_=st_ps)
                nc.sync.dma_start(out=st_d[:, :], in_=st_sb)

    nc.compile()  # bacc lowering: splits >1-wait instructions for the TRN2 ISA
    return nc


def make_in_map(o_slice, tgt_slice, img_slice, n_macros: int = N_MACROS, gpm: int = GPM):
    """Host-side pre-layout for one core.

    Pixel q = m*(128*gpm) + p*gpm + j  ->  macro m, partition p, group j.
    comb[m*128+p, j*K + 0:100]   = o[q]        (bf16)
    comb[m*128+p, j*K + 100:103] = img[q]      (bf16)
    comb[m*128+p, j*K + 103:106] = img[q]^2    (bf16)
    tgtf[p, m*gpm+j] = target[q]               (bf16; labels < 256 exact)
    iotarep[p, l*gpm+j] = l                    (bf16 const)
    """
    n_pix = 128 * gpm * n_macros
    assert o_slice.shape == (n_pix, L)

    o4 = np.asarray(o_slice, dtype=np.float32).reshape(n_macros, 128, gpm, L)
    img4 = np.asarray(img_slice, dtype=np.float32).reshape(n_macros, 128, gpm, 3)
    comb = np.empty((n_macros, 128, gpm, K), dtype=np.float32)
    comb[..., 0:L] = o4
    comb[..., L : L + 3] = img4
    comb[..., L + 3 : K] = img4 * img4

    t = np.asarray(tgt_slice).reshape(n_macros, 128, gpm)
    tgt_pre = np.ascontiguousarray(t.transpose(1, 0, 2)).reshape(128, n_macros * gpm)

    iotarep = np.broadcast_to(
        np.repeat(np.arange(L, dtype=np.float32), gpm), (128, L * gpm)
    )
    return {
        "comb": comb.astype(NP_BF16).reshape(n_macros * 128, gpm * K),
        "iotarep": np.ascontiguousarray(iotarep.astype(NP_BF16)),
        "tgtf": np.ascontiguousarray(tgt_pre.astype(NP_BF16)),
    }


def finalize(results, n_pix_total=HW):
    """Combine per-core partial results (host-side unshard) into the scalar loss."""
    lse_sum = 0.0
    gather_sum = 0.0
    s1 = np.zeros((L, 3), dtype=np.float64)
    s2 = np.zeros((L, 3), dtype=np.float64)
    for r in results:
        lse_sum += float(np.sum(np.asarray(r["lse_out"], dtype=np.float64)))
        st = np.asarray(r["st_out"], dtype=np.float64)  # [106, 100]
        gather_sum += float(np.trace(st[0:L, 0:L]))
        s1 += st[L : L + 3, :].T
        s2 += st[L + 3 : K, :].T
    loss1 = (lse_sum - gather_sum) / n_pix_total
    mean = s1 / n_pix_total
    var = np.maximum(s2 / n_pix_total - mean * mean, 0.0)
    std_all = float(np.sum(np.sqrt(var)))
    loss2 = 1.0 / (1.0 + np.exp(-std_all))
    return np.float32((1.0 - ALPHA_SAL) * loss1 + ALPHA_SAL * loss2)


_NC_CACHE = {}


def _get_nc():
    if "nc" not in _NC_CACHE:
        _NC_CACHE["nc"] = build_nc()
    return _NC_CACHE["nc"]


def kernel(output, target, img):
    output = np.asarray(output, dtype=np.float32)
    target = np.asarray(target)
    img = np.asarray(img, dtype=np.float32)
    assert output.shape == (HW, L)
    img_flat = img.reshape(HW, 3)

    in_maps = []
    for c in range(N_CORES):
        lo, hi = c * PIX_PER_CORE, (c + 1) * PIX_PER_CORE
        in_maps.append(
            make_in_map(output[lo:hi], target[lo:hi], img_flat[lo:hi])
        )

    nc = _get_nc()
    res = run_bass_kernel_spmd(nc, in_maps, core_ids=list(range(N_CORES)))
    return finalize(res.results)


if __name__ == "__main__":
    nc = build_nc(n_macros=1)
    print("built ok:", len(nc.inst_map), "instructions")
